# revision 12
# baseline (speedup 1.0000x reference)
"""BFP8 block quantize-dequantize for Trainium2 (Bass/Tile), 8-core data parallel.

Problem: x (8, 4096, 4096) f32. Each contiguous block of 16 elements (along the
flattened last dims) shares an exponent e = floor(log2(max|x|)); values are
quantized to signed 8-bit mantissas at scale 2^(e-7) and dequantized back.

Sharding: pure data parallel on the leading axis — core c processes x[c].

Per-core kernel (memory-bound; HBM roofline ~358 GB/s/core):
  - The dequantized values q * 2^(e-7) with q in [-128, 127] have at most 8
    significant bits, so every output value is EXACTLY representable in
    bfloat16. The device writes bf16 (32 MiB instead of 64 MiB) and the host
    upcasts to f32 losslessly: HBM traffic is 96 MiB/core (~281 us floor).
  - With DMA at ~4.4 us per steady tile, no single engine may exceed that, so
    the work is spread over all four compute engines:
      DVE:    block abs-max reduce; exponent bit-math (scale/rcp/bf16-scale
              are exact bit tricks on the f32 exponent field); final dequant
              qb * s16 all-bf16 stride-1 -> 2x DVE perf mode.
      GPSIMD: y = x * rcp (f32; GPSIMD cannot write int8 from float inputs,
              so the scaling multiply and the round are split).
      ACT:    q = int8(y) — the f32->int8 output conversion is the round-to-
              nearest-even + clamp to [-128,127]; qb = bf16(q) (exact);
              broadcast-materialization of the per-block bf16 scale (high
              half of the f32 scale bits, exact for powers of two); issues
              the store DMA (separate HWDGE ring from loads).
Zero/denormal blocks: expb clamps to 0 -> scale 0 -> out exactly 0.
"""
import numpy as np

try:
    import concourse.bacc as bacc
except ImportError:  # pragma: no cover - fallback for bare environments
    import sys
    for _p in ("/opt/trn_rl_repo", "/root/.axon_site/_ro/trn_rl_repo"):
        if _p not in sys.path:
            sys.path.insert(0, _p)
    import concourse.bacc as bacc
import concourse.mybir as mybir
import concourse.tile as tile
from concourse.bass_utils import run_bass_kernel_spmd

N_CORES = 8
P = 128                      # SBUF partitions
ROWS, COLS = 4096, 4096      # per-core shard
BLK = 16                     # elements sharing one exponent
MBITS_M1 = 7                 # mantissa_bits - 1
EXP_MASK = 0x7F800000

TILE_F = 2048                # f32 elements per partition per steady-state tile
TAPER_N, TAPER_F = 2, 1024   # smaller tiles at each end: faster pipeline fill/drain
BUFS = 6


def _schedule():
    total_f = ROWS * COLS // P
    end = TAPER_N * TAPER_F
    mid = total_f - 2 * end
    assert mid % TILE_F == 0
    return [TAPER_F] * TAPER_N + [TILE_F] * (mid // TILE_F) + [TAPER_F] * TAPER_N


def build(reps=1):
    nc = bacc.Bacc()
    x = nc.dram_tensor("x", [ROWS, COLS], mybir.dt.float32, kind="ExternalInput")
    out = nc.dram_tensor("out", [ROWS, COLS], mybir.dt.bfloat16, kind="ExternalOutput")

    sched = _schedule()
    offs = [0]
    for f in sched:
        offs.append(offs[-1] + P * f)
    assert offs[-1] == ROWS * COLS
    xflat = x[:].rearrange("r c -> (r c)")
    outflat = out[:].rearrange("r c -> (r c)")

    with tile.TileContext(nc) as tc:
        with tc.tile_pool(name="sbuf", bufs=BUFS) as pool:
            for t, f in [(t, f) for _ in range(reps) for t, f in enumerate(sched)]:
                nb = f // BLK
                xt = pool.tile([P, f], mybir.dt.float32, tag="x")
                nc.sync.dma_start(xt[:], xflat[offs[t]:offs[t + 1]].rearrange("(p f) -> p f", p=P))
                x3 = xt[:].rearrange("p (b k) -> p b k", k=BLK)

                # block max|x|  (DVE reduce)
                bmax = pool.tile([P, nb], mybir.dt.float32, tag="bmax")
                nc.vector.tensor_reduce(
                    bmax[:], x3, axis=mybir.AxisListType.X,
                    op=mybir.AluOpType.max, apply_absolute_value=True,
                )
                # expb = exponent field of bmax == bits of 2^e
                expb = pool.tile([P, nb], mybir.dt.int32, tag="expb")
                nc.vector.tensor_scalar(
                    expb[:], bmax[:].bitcast(mybir.dt.int32),
                    scalar1=EXP_MASK, scalar2=None,
                    op0=mybir.AluOpType.bitwise_and,
                )
                # scale_bits = max(expb, 7<<23) - (7<<23)   [= 2^(e-7); 0 for zero/denormal blocks]
                scaleb = pool.tile([P, nb], mybir.dt.int32, tag="scaleb")
                nc.vector.tensor_scalar(
                    scaleb[:], expb[:],
                    scalar1=(MBITS_M1 << 23), scalar2=-(MBITS_M1 << 23),
                    op0=mybir.AluOpType.max, op1=mybir.AluOpType.add,
                )
                # rcp_bits = (254<<23) - scale_bits         [= 2^(7-e)]
                rcpb = pool.tile([P, nb], mybir.dt.int32, tag="rcpb")
                nc.vector.tensor_scalar(
                    rcpb[:], scaleb[:], scalar1=-1, scalar2=(254 << 23),
                    op0=mybir.AluOpType.mult, op1=mybir.AluOpType.add,
                )
                rcp_b = rcpb[:].bitcast(mybir.dt.float32).unsqueeze(2).broadcast_to((P, nb, BLK))
                # bf16 scale bits = high 16-bit half of the f32 scale bits
                # (exact: scale is a power of two) — addressed as a stride-2 view
                sc16_b = (scaleb[:].bitcast(mybir.dt.bfloat16)
                          .rearrange("p (b two) -> p b two", two=2)[:, :, 1:2]
                          .broadcast_to((P, nb, BLK)))

                # y = x * rcp  (GPSIMD, f32)
                yt = pool.tile([P, f], mybir.dt.float32, tag="y")
                nc.gpsimd.tensor_tensor(
                    yt[:].rearrange("p (b k) -> p b k", k=BLK),
                    x3, rcp_b, op=mybir.AluOpType.mult,
                )
                # q = sat_int8(round(y)) == clip(round(x / scale), -128, 127)  — ACT
                q = pool.tile([P, f], mybir.dt.int8, tag="q")
                nc.scalar.copy(q[:], yt[:])
                # qb = bf16(q)  (exact: |q| <= 128)  — ACT
                qb = pool.tile([P, f], mybir.dt.bfloat16, tag="qb")
                nc.scalar.copy(qb[:], q[:])
                # s16[p, b*16+k] = bf16 scale of block b  — ACT broadcast
                s16 = pool.tile([P, f], mybir.dt.bfloat16, tag="s16")
                nc.scalar.copy(
                    s16[:].rearrange("p (b k) -> p b k", k=BLK), sc16_b,
                )
                # out = qb * s16 (all bf16, stride-1 -> DVE 2x mode; exact in bf16)
                deq = pool.tile([P, f], mybir.dt.bfloat16, tag="deq")
                nc.vector.tensor_tensor(deq[:], qb[:], s16[:], op=mybir.AluOpType.mult)
                nc.scalar.dma_start(
                    outflat[offs[t]:offs[t + 1]].rearrange("(p f) -> p f", p=P), deq[:])
    nc.finalize()
    return nc


_NC_CACHE = {}


def _get_nc(reps=1):
    if reps not in _NC_CACHE:
        _NC_CACHE[reps] = build(reps)
    return _NC_CACHE[reps]


def kernel(x: np.ndarray) -> np.ndarray:
    x = np.asarray(x)
    assert x.shape == (N_CORES, ROWS, COLS) and x.dtype == np.float32, (x.shape, x.dtype)
    nc = _get_nc()
    in_maps = [{"x": np.ascontiguousarray(x[c])} for c in range(N_CORES)]
    res = run_bass_kernel_spmd(nc, in_maps, core_ids=list(range(N_CORES)))
    return np.stack([np.asarray(r["out"]).astype(np.float32) for r in res.results], axis=0)


# revision 13
# speedup vs baseline: 1.4664x; 1.4664x over previous
"""BFP8 block quantize-dequantize for Trainium2 (Bass/Tile), 8-core data parallel.

Output written as bf16 (exact; dequantized values have <= 8 significant bits)
-> 96 MiB HBM traffic/core (~281 us floor at 358 GB/s).

Measured per-4096-elem-tile engine op costs (HW): DVE 1x tt/reduce 4.33 us,
DVE bf16 2x tt 2.06 us, GPSIMD big tt ~10.8-11.1 us, ACT activation 3.5 us,
DMA window 8.8 us. No engine may exceed the DMA window, so quant and dequant
are split fractionally:
  quant (x*rcp -> sat int8):   DVE 45/128 direct; GPSIMD computes y=x*rcp f32
                               for 83/128 and ACT converts f32->int8 (round-
                               to-nearest-even + clamp for free).
  dequant (q*scale -> bf16):   DVE 1x int8-path 32/128; DVE 2x bf16-path
                               77/128 (ACT materializes qb=bf16(q) and the
                               stride-1 bf16 scale for those blocks); GPSIMD
                               19/128.
  scale/rcp/bf16-scale are exact bit tricks on the f32 exponent field.
Post-quant ops run with a 1-tile software skew and stores with a 2-tile skew
so no engine queue head-blocks on a cross-engine dependency.
Zero/denormal blocks: expb clamps to 0 -> scale 0 -> out exactly 0.
"""
import numpy as np

try:
    import concourse.bacc as bacc
except ImportError:  # pragma: no cover - fallback for bare environments
    import sys
    for _p in ("/opt/trn_rl_repo", "/root/.axon_site/_ro/trn_rl_repo"):
        if _p not in sys.path:
            sys.path.insert(0, _p)
    import concourse.bacc as bacc
import concourse.mybir as mybir
import concourse.tile as tile
from concourse.bass_utils import run_bass_kernel_spmd

N_CORES = 8
P = 128
ROWS, COLS = 4096, 4096
BLK = 16
MBITS_M1 = 7
EXP_MASK = 0x7F800000

TILE_F = 4096
TAPER_N, TAPER_F = 2, 1024
BUFS = 3
# block-count splits, as fractions num/128 of each tile's blocks
QA = 45        # quant on DVE directly
D1 = 32        # dequant on DVE via 1x int8 path
D2 = 77        # dequant on DVE via 2x bf16 path (ACT-converted operands)
               # remainder (128 - D1 - D2 = 19) on GPSIMD


def _schedule():
    total_f = ROWS * COLS // P
    end = TAPER_N * TAPER_F
    mid = total_f - 2 * end
    assert mid % TILE_F == 0
    return [TAPER_F] * TAPER_N + [TILE_F] * (mid // TILE_F) + [TAPER_F] * TAPER_N


def build(reps=1):
    nc = bacc.Bacc()
    x = nc.dram_tensor("x", [ROWS, COLS], mybir.dt.float32, kind="ExternalInput")
    out = nc.dram_tensor("out", [ROWS, COLS], mybir.dt.bfloat16, kind="ExternalOutput")

    sched = _schedule()
    offs = [0]
    for f in sched:
        offs.append(offs[-1] + P * f)
    assert offs[-1] == ROWS * COLS
    xflat = x[:].rearrange("r c -> (r c)")
    outflat = out[:].rearrange("r c -> (r c)")

    steps = [(t, f) for _ in range(reps) for t, f in enumerate(sched)]
    T = len(steps)

    with tile.TileContext(nc) as tc:
        with tc.tile_pool(name="sbuf", bufs=BUFS) as pool:
            st1, st2 = {}, {}

            def stage1(i_step, t, f):
                nb = f // BLK
                ba = nb * QA // 128
                fa = ba * BLK
                xt = pool.tile([P, f], mybir.dt.float32, tag="x")
                nc.sync.dma_start(xt[:], xflat[offs[t]:offs[t + 1]].rearrange("(p f) -> p f", p=P))
                x3 = xt[:].rearrange("p (b k) -> p b k", k=BLK)

                bmax = pool.tile([P, nb], mybir.dt.float32, tag="bmax")
                nc.vector.tensor_reduce(
                    bmax[:], x3, axis=mybir.AxisListType.X,
                    op=mybir.AluOpType.max, apply_absolute_value=True,
                )
                expb = pool.tile([P, nb], mybir.dt.int32, tag="expb")
                nc.vector.tensor_scalar(
                    expb[:], bmax[:].bitcast(mybir.dt.int32),
                    scalar1=EXP_MASK, scalar2=None, op0=mybir.AluOpType.bitwise_and,
                )
                scaleb = pool.tile([P, nb], mybir.dt.int32, tag="scaleb")
                nc.vector.tensor_scalar(
                    scaleb[:], expb[:],
                    scalar1=(MBITS_M1 << 23), scalar2=-(MBITS_M1 << 23),
                    op0=mybir.AluOpType.max, op1=mybir.AluOpType.add,
                )
                rcpb = pool.tile([P, nb], mybir.dt.int32, tag="rcpb")
                nc.vector.tensor_scalar(
                    rcpb[:], scaleb[:], scalar1=-1, scalar2=(254 << 23),
                    op0=mybir.AluOpType.mult, op1=mybir.AluOpType.add,
                )
                rcp_b = rcpb[:].bitcast(mybir.dt.float32).unsqueeze(2).broadcast_to((P, nb, BLK))

                q = pool.tile([P, f], mybir.dt.int8, tag="q")
                q3 = q[:].rearrange("p (b k) -> p b k", k=BLK)
                # quant A-part on DVE (f32->int8 conversion rounds + clamps)
                if ba > 0:
                    nc.vector.tensor_tensor(
                        q3[:, :ba], x3[:, :ba], rcp_b[:, :ba], op=mybir.AluOpType.mult,
                    )
                # quant rest: GPSIMD y = x*rcp (f32), ACT converts f32 -> int8
                yt = pool.tile([P, f - fa], mybir.dt.float32, tag="y")
                nc.gpsimd.tensor_tensor(
                    yt[:].rearrange("p (b k) -> p b k", k=BLK),
                    x3[:, ba:], rcp_b[:, ba:], op=mybir.AluOpType.mult,
                )
                nc.scalar.copy(q[:, fa:], yt[:])
                st1[i_step] = (t, f, q, scaleb)

            def stage2(i_step):
                t, f, q, scaleb = st1.pop(i_step)
                nb = f // BLK
                b1 = nb * D1 // 128
                b2 = nb * D2 // 128
                q3 = q[:].rearrange("p (b k) -> p b k", k=BLK)
                sc_b = (scaleb[:].bitcast(mybir.dt.float32)
                        .unsqueeze(2).broadcast_to((P, nb, BLK)))
                deq = pool.tile([P, f], mybir.dt.bfloat16, tag="deq")
                d3 = deq[:].rearrange("p (b k) -> p b k", k=BLK)
                # DVE 1x int8 path
                if b1 > 0:
                    nc.vector.tensor_tensor(
                        d3[:, :b1], q3[:, :b1], sc_b[:, :b1], op=mybir.AluOpType.mult,
                    )
                # DVE 2x bf16 path: ACT converts q -> bf16 and materializes the
                # stride-1 bf16 scale (high half of f32 scale bits; exact).
                if b2 > 0:
                    qb = pool.tile([P, b2 * BLK], mybir.dt.bfloat16, tag="qb")
                    nc.scalar.copy(qb[:], q[:, b1 * BLK:(b1 + b2) * BLK])
                    s16 = pool.tile([P, b2 * BLK], mybir.dt.bfloat16, tag="s16")
                    sc16_b = (scaleb[:].bitcast(mybir.dt.bfloat16)
                              .rearrange("p (b two) -> p b two", two=2)[:, b1:b1 + b2, 1:2]
                              .broadcast_to((P, b2, BLK)))
                    nc.scalar.copy(s16[:].rearrange("p (b k) -> p b k", k=BLK), sc16_b)
                    nc.vector.tensor_tensor(
                        deq[:, b1 * BLK:(b1 + b2) * BLK], qb[:], s16[:],
                        op=mybir.AluOpType.mult,
                    )
                # GPSIMD path for the remainder
                if b1 + b2 < nb:
                    nc.gpsimd.tensor_tensor(
                        d3[:, b1 + b2:], q3[:, b1 + b2:], sc_b[:, b1 + b2:],
                        op=mybir.AluOpType.mult,
                    )
                st2[i_step] = (t, f, deq)

            def stage3(i_step):
                t, f, deq = st2.pop(i_step)
                eng = nc.sync if (i_step % 2 == 0) else nc.scalar
                eng.dma_start(
                    outflat[offs[t]:offs[t + 1]].rearrange("(p f) -> p f", p=P), deq[:])

            for i in range(T + 2):
                if i < T:
                    stage1(i, *steps[i])
                if 1 <= i < T + 1:
                    stage2(i - 1)
                if i >= 2:
                    stage3(i - 2)
    nc.finalize()
    return nc


_NC_CACHE = {}


def _get_nc(reps=1):
    if reps not in _NC_CACHE:
        _NC_CACHE[reps] = build(reps)
    return _NC_CACHE[reps]


def kernel(x: np.ndarray) -> np.ndarray:
    x = np.asarray(x)
    assert x.shape == (N_CORES, ROWS, COLS) and x.dtype == np.float32, (x.shape, x.dtype)
    nc = _get_nc()
    in_maps = [{"x": np.ascontiguousarray(x[c])} for c in range(N_CORES)]
    res = run_bass_kernel_spmd(nc, in_maps, core_ids=list(range(N_CORES)))
    return np.stack([np.asarray(r["out"]).astype(np.float32) for r in res.results], axis=0)
